# revision 17
# baseline (speedup 1.0000x reference)
"""Canny on 8 trn2 cores — transposed-native layout (v7).

Sharding: the padded image is split into 18 self-contained column-chunks
of 128 cols (118 owned + 2x5 duplicated halo).  Each core runs an
identical SPMD program over: chunk A (full height), chunk B (full
height), chunk C (a 522-row slice of one of the two leftover chunks).
8 cores x (2 + 1/4) = 18 chunk-equivalents.  No cross-chunk/core
communication: all halos are host-duplicated.

Per chunk (cols on partitions, rows in free dim):
  DVE : vertical gaussian + vertical sobel parts (free-dim stencils),
        hi/lo splits, r2, channel sums, NMS compare/select, hysteresis
        row-max.
  PE  : horizontal 7-tap convs as band matmuls (bf16 hi/lo, 3 passes),
        NMS col-shifts of g (exact bf16 shift matmuls), Band3 col-max.
  ACT : hi-copies, squares off PSUM, sqrt, abs, shift evacuations.
"""

import numpy as np
import ml_dtypes

_COMPILED = {}

H = 2048
W = 2048
OWN = 118                 # owned cols per chunk
NCHUNK = 18               # ceil(2048 / 118)
PADR = H + 10             # padded rows
PADC = OWN * (NCHUNK - 1) + 128   # padded cols (>= 2048 + 10)
FD_A = PADR               # full-height chunk input rows
SLICE = 512               # owned rows per C-slice
FD_C = SLICE + 10

g5 = np.exp(-0.5 * (np.arange(5) - 2.0) ** 2)
GA, GB = float(np.float32(g5[0])), float(np.float32(g5[1]))
_hx = np.convolve(g5, [1, 0, -1]).astype(np.float64)
_hy = np.convolve(g5, [1, 2, 1]).astype(np.float64)


def _band(k7):
    Wm = np.zeros((128, 128), np.float64)
    for m in range(128):
        for dk in range(-3, 4):
            k = m + dk
            if 0 <= k < 128:
                Wm[k, m] = k7[dk + 3]
    return Wm


def _hilo(Wd):
    hi = Wd.astype(ml_dtypes.bfloat16)
    lo = (Wd - hi.astype(np.float64)).astype(ml_dtypes.bfloat16)
    return np.asarray(hi), np.asarray(lo)


def _weights():
    Wxh, Wxl = _hilo(_band(_hx))
    Wyh, Wyl = _hilo(_band(_hy))
    sR = np.zeros((128, 128), np.float64)
    sL = np.zeros((128, 128), np.float64)
    for m in range(127):
        sR[m + 1, m] = 1.0
        sL[m, m + 1] = 1.0
    b3 = np.zeros((128, 128), np.float64)
    for m in range(128):
        for dk in (-1, 0, 1):
            if 0 <= m + dk < 128:
                b3[m + dk, m] = 1.0
    return {"wxh": Wxh, "wxl": Wxl, "wyh": Wyh, "wyl": Wyl,
            "srd": np.asarray(sR.astype(ml_dtypes.bfloat16)),
            "sld": np.asarray(sL.astype(ml_dtypes.bfloat16)),
            "b3d": b3.astype(np.float16)}


def _build(low, high):
    import concourse.bass as bass
    import concourse.bacc as bacc
    import concourse.mybir as mybir
    from concourse.tile import TileContext

    f32 = mybir.dt.float32
    bf16 = mybir.dt.bfloat16
    f16 = mybir.dt.float16
    u8 = mybir.dt.uint8
    Alu = mybir.AluOpType
    Act = mybir.ActivationFunctionType

    t1c = float(np.float32(np.tan(np.deg2rad(np.float64(22.5)))))
    t2c = float(np.float32(np.tan(np.deg2rad(np.float64(67.5)))))
    lowx = float(np.nextafter(np.float32(low), np.float32(0.0)))
    high_f = float(high)

    nc = bacc.Bacc()
    xA = nc.dram_tensor("xA", [3, 128, FD_A], f32, kind="ExternalInput")
    xB = nc.dram_tensor("xB", [3, 128, FD_A], f32, kind="ExternalInput")
    xC = nc.dram_tensor("xC", [3, 128, FD_C], f32, kind="ExternalInput")
    wdr = {}
    for nm in ("wxh", "wxl", "wyh", "wyl", "srd", "sld"):
        wdr[nm] = nc.dram_tensor(nm, [128, 128], bf16, kind="ExternalInput")
    wdr["b3d"] = nc.dram_tensor("b3d", [128, 128], f16, kind="ExternalInput")
    oA = nc.dram_tensor("oA", [128, FD_A - 10], f16, kind="ExternalOutput")
    oB = nc.dram_tensor("oB", [128, FD_A - 10], f16, kind="ExternalOutput")
    oC = nc.dram_tensor("oC", [128, FD_C - 10], f16, kind="ExternalOutput")

    with TileContext(nc) as tc:
        with tc.tile_pool(name="sb", bufs=1) as pool, \
             tc.tile_pool(name="ps", bufs=2, space="PSUM") as psp:
            wt = {}
            for nm in ("wxh", "wxl", "wyh", "wyl", "srd", "sld"):
                t = pool.tile([128, 128], bf16, tag=nm)
                nc.sync.dma_start(out=t[:], in_=bass.AP(wdr[nm], 0, [[128, 128], [1, 128]]))
                wt[nm] = t
            b3t = pool.tile([128, 128], f16, tag="b3")
            nc.sync.dma_start(out=b3t[:], in_=bass.AP(wdr["b3d"], 0, [[128, 128], [1, 128]]))

            for xdr, odr, NR in ((xA, oA, FD_A), (xB, oB, FD_A), (xC, oC, FD_C)):
                RV = NR - 4
                RT = NR - 6
                gpl = pool.tile([128, FD_A - 6], f32, tag="g")
                sgx = pool.tile([128, FD_A - 6], f32, tag="sgx")
                sgy = pool.tile([128, FD_A - 6], f32, tag="sgy")

                for c in range(3):
                    img = pool.tile([128, FD_A], f32, tag=f"img{c % 2}")
                    nc.sync.dma_start(out=img[:, 0:NR],
                                      in_=bass.AP(xdr, c * 128 * NR, [[NR, 128], [1, NR]]))
                    v1 = pool.tile([128, FD_A - 4], f32, tag="tA")
                    v2 = pool.tile([128, FD_A - 4], f32, tag="tB")
                    vb1 = pool.tile([128, FD_A - 4], f32, tag="tC")
                    vb = pool.tile([128, FD_A - 4], f32, tag="tD")
                    nc.vector.tensor_tensor(v1[:, 0:RV], img[:, 1:RV + 1], img[:, 3:RV + 3], Alu.add)
                    nc.vector.tensor_tensor(v2[:, 0:RV], img[:, 0:RV], img[:, 4:RV + 4], Alu.add)
                    nc.vector.scalar_tensor_tensor(
                        vb1[:, 0:RV], v1[:, 0:RV], GB, img[:, 2:RV + 2], Alu.mult, Alu.add)
                    nc.vector.scalar_tensor_tensor(
                        vb[:, 0:RV], v2[:, 0:RV], GA, vb1[:, 0:RV], Alu.mult, Alu.add)

                    u = pool.tile([128, FD_A - 6], f32, tag="tA")
                    t1 = pool.tile([128, FD_A - 6], f32, tag="tB")
                    t2 = pool.tile([128, FD_A - 6], f32, tag="tC")
                    nc.vector.tensor_tensor(u[:, 0:RT], vb[:, 0:RT], vb[:, 2:RT + 2], Alu.add)
                    nc.vector.scalar_tensor_tensor(
                        t1[:, 0:RT], vb[:, 1:RT + 1], 2.0, u[:, 0:RT], Alu.mult, Alu.add)
                    nc.vector.tensor_tensor(t2[:, 0:RT], vb[:, 0:RT], vb[:, 2:RT + 2], Alu.subtract)

                    t1h = pool.tile([128, FD_A - 6], bf16, tag="t1h")
                    t1l = pool.tile([128, FD_A - 6], bf16, tag="t1l")
                    t2h = pool.tile([128, FD_A - 6], bf16, tag="t2h")
                    t2l = pool.tile([128, FD_A - 6], bf16, tag="t2l")
                    nc.scalar.activation(t1h[:, 0:RT], t1[:, 0:RT], Act.Copy)
                    nc.vector.tensor_tensor(t1l[:, 0:RT], t1[:, 0:RT], t1h[:, 0:RT], Alu.subtract)
                    nc.scalar.activation(t2h[:, 0:RT], t2[:, 0:RT], Act.Copy)
                    nc.vector.tensor_tensor(t2l[:, 0:RT], t2[:, 0:RT], t2h[:, 0:RT], Alu.subtract)

                    q1 = pool.tile([128, FD_A - 6], f32, tag="tA")
                    q2 = pool.tile([128, FD_A - 6], f32, tag="tD")
                    for s0 in range(0, RT, 512):
                        s1_ = min(s0 + 512, RT)
                        w = s1_ - s0
                        gxs = psp.tile([128, 512], f32, tag="px")
                        gys = psp.tile([128, 512], f32, tag="py")
                        nc.tensor.matmul(gxs[:, 0:w], wt["wxh"][:], t1h[:, s0:s1_], start=True, stop=False)
                        nc.tensor.matmul(gxs[:, 0:w], wt["wxh"][:], t1l[:, s0:s1_], start=False, stop=False)
                        nc.tensor.matmul(gxs[:, 0:w], wt["wxl"][:], t1h[:, s0:s1_], start=False, stop=True)
                        nc.tensor.matmul(gys[:, 0:w], wt["wyh"][:], t2h[:, s0:s1_], start=True, stop=False)
                        nc.tensor.matmul(gys[:, 0:w], wt["wyh"][:], t2l[:, s0:s1_], start=False, stop=False)
                        nc.tensor.matmul(gys[:, 0:w], wt["wyl"][:], t2h[:, s0:s1_], start=False, stop=True)
                        nc.scalar.activation(q1[:, s0:s1_], gxs[:, 0:w], Act.Square)
                        nc.scalar.activation(q2[:, s0:s1_], gys[:, 0:w], Act.Square)
                        if c == 0:
                            nc.vector.tensor_copy(sgx[:, s0:s1_], gxs[:, 0:w])
                            nc.vector.tensor_copy(sgy[:, s0:s1_], gys[:, 0:w])
                        else:
                            nc.vector.tensor_tensor(sgx[:, s0:s1_], sgx[:, s0:s1_], gxs[:, 0:w], Alu.add)
                            nc.vector.tensor_tensor(sgy[:, s0:s1_], sgy[:, s0:s1_], gys[:, 0:w], Alu.add)

                    r2 = pool.tile([128, FD_A - 6], f32, tag="tB")
                    m = pool.tile([128, FD_A - 6], f32, tag="tC")
                    nc.vector.tensor_tensor(r2[:, 0:RT], q1[:, 0:RT], q2[:, 0:RT], Alu.add)
                    nc.scalar.activation(m[:, 0:RT], r2[:, 0:RT], Act.Sqrt)
                    if c == 0:
                        nc.vector.tensor_copy(gpl[:, 0:RT], m[:, 0:RT])
                    else:
                        nc.vector.tensor_tensor(gpl[:, 0:RT], gpl[:, 0:RT], m[:, 0:RT], Alu.add)

                # ---- NMS + hysteresis ----
                ghi = pool.tile([128, FD_A - 6], bf16, tag="t1h")
                glo = pool.tile([128, FD_A - 6], bf16, tag="t1l")
                nc.scalar.activation(ghi[:, 0:RT], gpl[:, 0:RT], Act.Copy)
                nc.vector.tensor_tensor(glo[:, 0:RT], gpl[:, 0:RT], ghi[:, 0:RT], Alu.subtract)
                gRs = pool.tile([128, FD_A - 6], f32, tag="gRs")
                gLs = pool.tile([128, FD_A - 6], f32, tag="gLs")
                for s0 in range(0, RT, 512):
                    s1_ = min(s0 + 512, RT)
                    w = s1_ - s0
                    pR = psp.tile([128, 512], f32, tag="px")
                    pL = psp.tile([128, 512], f32, tag="py")
                    nc.tensor.matmul(pR[:, 0:w], wt["srd"][:], ghi[:, s0:s1_], start=True, stop=False)
                    nc.tensor.matmul(pR[:, 0:w], wt["srd"][:], glo[:, s0:s1_], start=False, stop=True)
                    nc.tensor.matmul(pL[:, 0:w], wt["sld"][:], ghi[:, s0:s1_], start=True, stop=False)
                    nc.tensor.matmul(pL[:, 0:w], wt["sld"][:], glo[:, s0:s1_], start=False, stop=True)
                    nc.scalar.activation(gRs[:, s0:s1_], pR[:, 0:w], Act.Copy)
                    nc.scalar.activation(gLs[:, s0:s1_], pL[:, 0:w], Act.Copy)

                RN = RT - 2
                rr = pool.tile([128, FD_A - 8], f32, tag="tA")
                ss = pool.tile([128, FD_A - 8], f32, tag="tB")
                m0 = pool.tile([128, FD_A - 8], u8, tag="mk0")
                m2 = pool.tile([128, FD_A - 8], u8, tag="mk1")
                d = pool.tile([128, FD_A - 8], f32, tag="tC")
                dpos = pool.tile([128, FD_A - 8], u8, tag="mk2")
                nc.scalar.activation(rr[:, 0:RN], sgy[:, 1:RN + 1], Act.Abs)
                nc.scalar.activation(ss[:, 0:RN], sgx[:, 1:RN + 1], Act.Abs)
                nc.vector.scalar_tensor_tensor(m0[:, 0:RN], ss[:, 0:RN], t1c, rr[:, 0:RN], Alu.mult, Alu.is_ge)
                nc.vector.scalar_tensor_tensor(m2[:, 0:RN], ss[:, 0:RN], t2c, rr[:, 0:RN], Alu.mult, Alu.is_le)
                nc.vector.tensor_tensor(d[:, 0:RN], sgx[:, 1:RN + 1], sgy[:, 1:RN + 1], Alu.mult)
                nc.vector.tensor_scalar(dpos[:, 0:RN], d[:, 0:RN], 0.0, None, Alu.is_ge)

                cand = pool.tile([128, FD_A - 8], f32, tag="tD")
                cc = pool.tile([128, FD_A - 8], f32, tag="cc")
                nc.vector.tensor_tensor(cand[:, 0:RN], gRs[:, 2:RT], gLs[:, 0:RN], Alu.max)
                nc.vector.tensor_tensor(cc[:, 0:RN], gLs[:, 2:RT], gRs[:, 0:RN], Alu.max)
                nc.vector.copy_predicated(cc[:, 0:RN], dpos[:, 0:RN], cand[:, 0:RN])
                cand2 = pool.tile([128, FD_A - 8], f32, tag="tA2")
                nc.vector.tensor_tensor(cand2[:, 0:RN], gpl[:, 2:RT], gpl[:, 0:RN], Alu.max)
                nc.vector.copy_predicated(cc[:, 0:RN], m2[:, 0:RN], cand2[:, 0:RN])
                cand3 = pool.tile([128, FD_A - 8], f32, tag="tB2")
                nc.vector.tensor_tensor(cand3[:, 0:RN], gLs[:, 1:RN + 1], gRs[:, 1:RN + 1], Alu.max)
                nc.vector.copy_predicated(cc[:, 0:RN], m0[:, 0:RN], cand3[:, 0:RN])

                hp = pool.tile([128, FD_A - 8], f16, tag="hp")
                lm = pool.tile([128, FD_A - 8], f16, tag="lm")
                nc.vector.scalar_tensor_tensor(
                    hp[:, 0:RN], cc[:, 0:RN], high_f, gpl[:, 1:RN + 1], Alu.max, Alu.is_lt)
                nc.vector.scalar_tensor_tensor(
                    lm[:, 0:RN], cc[:, 0:RN], lowx, gpl[:, 1:RN + 1], Alu.max, Alu.is_lt)

                NO = RN - 2
                rm1 = pool.tile([128, FD_A - 10], f16, tag="rm1")
                rm = pool.tile([128, FD_A - 10], f16, tag="rm")
                nc.vector.tensor_tensor(rm1[:, 0:NO], hp[:, 0:NO], hp[:, 2:RN], Alu.max)
                nc.vector.tensor_tensor(rm[:, 0:NO], rm1[:, 0:NO], hp[:, 1:NO + 1], Alu.max)

                outt = pool.tile([128, FD_A - 10], f16, tag="outt")
                for s0 in range(0, NO, 512):
                    s1_ = min(s0 + 512, NO)
                    w = s1_ - s0
                    pc = psp.tile([128, 512], f32, tag="px")
                    nc.tensor.matmul(pc[:, 0:w], b3t[:], rm[:, s0:s1_], start=True, stop=True)
                    nc.vector.scalar_tensor_tensor(
                        outt[:, s0:s1_], pc[:, 0:w], 0.0, lm[:, 1 + s0:1 + s1_],
                        Alu.is_gt, Alu.mult)
                nc.sync.dma_start(out=bass.AP(odr, 0, [[NO, 128], [1, NO]]),
                                  in_=outt[:, 0:NO])
    nc.finalize()
    return nc


def _get_compiled(low, high):
    key = (low, high)
    if key not in _COMPILED:
        _COMPILED[key] = _build(low, high)
    return _COMPILED[key]


def kernel(img, threshold1, threshold2, _trace=False):
    from concourse import bass_utils

    t1 = float(np.asarray(threshold1))
    t2 = float(np.asarray(threshold2))
    low, high = min(t1, t2), max(t1, t2)

    x = np.ascontiguousarray(np.asarray(img, dtype=np.float32)[0])  # [3,H,W]
    xp = np.zeros((3, PADR, PADC), dtype=np.float32)
    xp[:, 5:5 + H, 5:5 + W] = x
    xt = np.ascontiguousarray(xp.transpose(0, 2, 1))  # [3, PADC cols, PADR rows]

    wts = _weights()
    in_maps = []
    for k in range(8):
        jA, jB = 2 * k, 2 * k + 1
        q = 16 + k // 4
        s = k % 4
        mp = dict(wts)
        mp["xA"] = np.ascontiguousarray(xt[:, OWN * jA:OWN * jA + 128, :])
        mp["xB"] = np.ascontiguousarray(xt[:, OWN * jB:OWN * jB + 128, :])
        mp["xC"] = np.ascontiguousarray(
            xt[:, OWN * q:OWN * q + 128, SLICE * s:SLICE * s + FD_C])
        in_maps.append(mp)

    nc = _get_compiled(low, high)
    res = bass_utils.run_bass_kernel_spmd(nc, in_maps, core_ids=list(range(8)),
                                          trace=_trace)

    full = np.zeros((1, 1, H, W), dtype=np.float32)
    for k in range(8):
        r = res.results[k]
        for j, key in ((2 * k, "oA"), (2 * k + 1, "oB")):
            c0 = OWN * j
            cw = min(OWN, W - c0)
            full[0, 0, :, c0:c0 + cw] = (
                r[key][5:5 + cw, :].T.astype(np.float32))
        q = 16 + k // 4
        s = k % 4
        c0 = OWN * q
        cw = min(OWN, W - c0)
        if cw > 0:
            full[0, 0, SLICE * s:SLICE * s + SLICE, c0:c0 + cw] = (
                r["oC"][5:5 + cw, :].T.astype(np.float32))
    full[:, :, 0, :] = 0.0
    full[:, :, -1, :] = 0.0
    full[:, :, :, 0] = 0.0
    full[:, :, :, -1] = 0.0
    if _trace:
        kernel._last_results = res
    return full
